# revision 1
# baseline (speedup 1.0000x reference)
"""BDCovpool + Triuvec kernel for Trainium2 (8 NeuronCores, data-parallel).

Math (per sample b, x[b]: [M=196, D=512], t: scalar):
  gram[i,j] = sum_m x[m,i] x[m,j]           (D x D)
  d[i]      = gram[i,i]
  dpre      = d[i] + d[j] - 2 gram
  dcov      = sqrt(exp(t) * relu(dpre) + 1e-5)
  cent      = dcov - rowmean - colmean + totmean   (dcov symmetric -> row==col)
  out       = upper triangle of cent, row-major (131328 per sample)

Device strategy per core (32 samples):
  PSUM_r = gram_chunk - (1/2) ones (x) d'  - gamma*I_blockdiag      (f32r matmuls)
     where d' = d - 196 (K-centering to keep f32r rounding small)
  dcov   = ACT Sqrt(PSUM * (-2 e^t) + bias_i), bias_i = e^t d'_i + 392 e^t + 1e-5
     accum_out gives row sums for free. gamma keeps the (noisy ~0) diagonal
     argument strictly positive; the known constant diagonal value is removed
     from the row-sum statistics on device and from the diagonal on the host.
  cent   = dcov - c_i - c_j,  c = rowsum/512 - mu/2 (mu = total mean)
  Only the upper-triangle row-blocks [128 x (512-128r)] are centered + written.
Host: shard B across 8 cores, assemble triu rows from the rectangular blocks,
subtract the constant gamma offset from the diagonal entries.
"""

import numpy as np

B, M, D = 256, 196, 512
NCORES = 8
S = B // NCORES  # samples per core
P = 128
NCH = D // P  # 4 row chunks
GAMMA = 256.0  # diagonal shift (bf16-exact); argument becomes ~2*gamma*e^t ~ 1.3
EPS = 1e-5
KC = float(M)  # diagonal centering constant: E[d] = M
# packed per-sample layout: 4 rectangles [128, 512-128r]
CH_W = [D - P * r for r in range(NCH)]
CH_OFF = [0]
for r in range(NCH - 1):
    CH_OFF.append(CH_OFF[-1] + P * CH_W[r])
OUT_PACK = CH_OFF[-1] + P * CH_W[-1]  # 163840
# row-interleaved device layout: out[s, p, :] = concat(chunk_r row p)
OO = [0]
for r in range(NCH - 1):
    OO.append(OO[-1] + CH_W[r])
OW = OO[-1] + CH_W[-1]  # 1280


def build_nc(n_samples=S, fixup=True):
    import concourse.bass as bass
    import concourse.mybir as mybir
    import concourse.tile as tile
    from concourse import library_config

    f32 = mybir.dt.float32
    f32r = mybir.dt.float32r
    bf16 = mybir.dt.bfloat16
    AF = mybir.ActivationFunctionType
    ALU = mybir.AluOpType

    nc = bass.Bass(
        "TRN2",
        target_bir_lowering=False,
        debug=False,
        enable_asserts=False,
    )

    x_d = nc.dram_tensor("x", [n_samples, M, D], f32r, kind="ExternalInput").ap()
    t_d = nc.dram_tensor("t", [1, 1], f32, kind="ExternalInput").ap()
    t128_d = nc.dram_tensor("t128", [P, 1], f32, kind="ExternalInput").ap()
    # host-provided constants: idc = [I(128) | -gamma*I(128)] bf16,
    # cc f32 [128,4]: col0,1 = -0.5 (diag-matmul lhsT), col2 = 1/(2*512^2)
    #                 (mu matmul lhsT), col3 = delta/1024 (rowsum fix)
    idc_d = nc.dram_tensor("idc", [P, 2 * P], bf16, kind="ExternalInput").ap()
    cc_d = nc.dram_tensor("cc", [P, 4], f32r, kind="ExternalInput").ap()
    ones_d = nc.dram_tensor("onesr", [1, P], f32r, kind="ExternalInput").ap()
    out_d = nc.dram_tensor("out", [n_samples, P, OW], f32, kind="ExternalOutput").ap()

    MB = M - P  # 68 rows in second k-chunk

    with tile.TileContext(nc) as tc:
        with (
            tc.tile_pool(name="const", bufs=1) as cpool,
            tc.tile_pool(name="xa", bufs=3) as xa_pool,
            tc.tile_pool(name="xb", bufs=3) as xb_pool,
            tc.tile_pool(name="sq", bufs=3) as sq_pool,
            tc.tile_pool(name="small", bufs=4) as sm_pool,
            tc.tile_pool(name="dcov", bufs=3 * NCH) as dc_pool,
            tc.tile_pool(name="uo", bufs=8) as uo_pool,
            tc.tile_pool(name="pg", bufs=5, space="PSUM") as pg_pool,
            tc.tile_pool(name="pnd", bufs=1, space="PSUM") as pnd_pool,
            tc.tile_pool(name="psm", bufs=1, space="PSUM") as psm_pool,
            tc.tile_pool(name="pr", bufs=1, space="PSUM") as pr_pool,
        ):
            # ---- once-per-kernel setup ----
            idc = cpool.tile([P, 2 * P], bf16, tag="idc")
            nc.sync.dma_start(idc[:], idc_d[:])
            cc = cpool.tile([P, 4], f32r, tag="cc")
            nc.sync.dma_start(cc[:], cc_d[:])
            onesr = cpool.tile([1, P], f32r, tag="onesr")
            nc.sync.dma_start(onesr[:], ones_d[:])
            tt = cpool.tile([1, 1], f32, tag="tt")
            nc.sync.dma_start(tt[:], t_d[:])
            t128 = cpool.tile([P, 1], f32, tag="t128")
            nc.sync.dma_start(t128[:], t128_d[:])
            et128 = cpool.tile([P, 1], f32, tag="et128")
            nc.scalar.activation(et128[:], t128[:], AF.Exp)
            scb = cpool.tile([P, 2], f32, tag="scb")
            # scb col0 = -2 e^t (sqrt scale); col1 = 392 e^t + 1e-5 (bias const)
            nc.vector.tensor_scalar_mul(scb[:, 0:1], et128[:], -2.0)
            nc.vector.tensor_scalar(
                scb[:, 1:2], et128[:], KC, EPS, ALU.mult, ALU.add
            )
            sc_ap = scb[:, 0:1]  # [128,1] = -2 e^t
            e392_ap = scb[:, 1:2]  # [128,1] = 392 e^t + 1e-5

            for s in range(n_samples):
                # ---- load x ----
                xa = xa_pool.tile([P, D], f32r, tag="xa")
                nc.sync.dma_start(xa[:], x_d[s, 0:P, :])
                xb = xb_pool.tile([MB, D], f32r, tag="xb")
                nc.sync.dma_start(xb[:], x_d[s, P:M, :])

                # ---- sq = x*x on gpsimd (idle engine) ----
                sqa = sq_pool.tile([P, D], f32r, tag="sqa")
                nc.gpsimd.tensor_mul(sqa[:], xa[:].bitcast(f32), xa[:].bitcast(f32))
                sqb = sq_pool.tile([MB, D], f32r, tag="sqb")
                nc.gpsimd.tensor_mul(sqb[:], xb[:].bitcast(f32), xb[:].bitcast(f32))

                # ---- -diag/2 via matmul with const -0.5 lhsT ----
                pb2 = pnd_pool.tile([1, D], f32, tag="pb2")
                pnd = pb2[0:1, :]
                nc.tensor.matmul(
                    pnd,
                    cc[:, 0:1],
                    sqa[:],
                    start=True,
                    stop=False,
                )
                nc.tensor.matmul(
                    pnd,
                    cc[0:MB, 0:1],
                    sqb[:],
                    start=False,
                    stop=True,
                )
                # negh = -d/2 + 98 = -(d-196)/2 = -d'/2
                negh = sm_pool.tile([1, D], f32r, tag="negh")
                nc.scalar.activation(negh[:], pnd, AF.Copy, bias=KC / 2.0)

                # bias columns via tiny column matmuls: ps[:,c] = -diag_c/2
                ps = psm_pool.tile([P, 12], f32, tag="ps")
                for c in range(NCH):
                    csl = slice(P * c, P * (c + 1))
                    nc.tensor.matmul(
                        ps[:, c : c + 1], sqa[:, csl].bitcast(f32),
                        cc[:, 0:1].bitcast(f32), start=True, stop=False,
                    )
                    nc.tensor.matmul(
                        ps[:, c : c + 1], sqb[:, csl].bitcast(f32),
                        cc[0:MB, 0:1].bitcast(f32), start=False, stop=True,
                    )
                bias4 = sm_pool.tile([P, NCH], f32, tag="bias4")
                nc.vector.tensor_scalar(
                    bias4[:], ps[:, 0:NCH].bitcast(f32), sc_ap, e392_ap,
                    ALU.mult, ALU.add,
                )

                # ---- gram chunks + fused corrections ----
                dcs = []
                for r in range(NCH):
                    pg = pg_pool.tile([P, D], f32, tag="pg")
                    sl = slice(P * r, P * (r + 1))
                    nc.tensor.matmul(
                        pg[:],
                        xa[:, sl],
                        xa[:],
                        start=True,
                        stop=False,
                    )
                    nc.tensor.matmul(
                        pg[:],
                        xb[:, sl],
                        xb[:],
                        start=False,
                        stop=False,
                    )
                    # - gamma I on the diagonal block
                    nc.tensor.matmul(
                        pg[:, sl],
                        idc[:, 0:P],
                        idc[:, P : 2 * P],
                        start=False,
                        stop=False,
                        skip_group_check=True,
                    )
                    # + ones (x) (-d'/2)
                    nc.tensor.matmul(
                        pg[:],
                        onesr[:],
                        negh[:],
                        start=False,
                        stop=True,
                    )
                    # dcov = sqrt(pg * (-2 e^t) + bias_i); accum -> row sums
                    dc = dc_pool.tile([P, D], f32r, tag="dcov")
                    nc.scalar.activation(
                        dc[:],
                        pg[:],
                        AF.Sqrt,
                        bias=bias4[:, r : r + 1],
                        scale=sc_ap,
                    )
                    dcs.append(dc)

                # ---- centering stats ----
                prv = pb2[0:1, :]
                for r in range(NCH):
                    nc.tensor.matmul(
                        prv,
                        cc[:, 1:2],
                        dcs[r][:],
                        start=(r == 0),
                        stop=(r == NCH - 1),
                    )
                cv0 = sm_pool.tile([1, D], f32, tag="cv0")
                muh = sm_pool.tile([1, 2], f32, tag="muh")
                nc.vector.tensor_scalar(
                    cv0[:], prv, 1.0 / D, 0.0, ALU.mult, ALU.add,
                    accum_out=muh[:, 0:1],
                )
                # mu_adj = sum(cv0)/1024 + delta/1024 (gamma'd diag removed)
                nc.vector.tensor_scalar(
                    muh[:, 1:2], muh[:, 0:1], 1.0 / (2.0 * D),
                    cc[0:1, 3:4].bitcast(f32), ALU.mult, ALU.add,
                )
                cvec = sm_pool.tile([1, D], f32r, tag="cvec")
                nc.vector.tensor_scalar(
                    cvec[:], cv0[:], muh[0:1, 1:2], None, ALU.subtract
                )
                # ccols = cvec transposed into columns via tiny rank-1s
                for c in range(NCH):
                    nc.tensor.matmul(
                        ps[:, c + 4 : c + 5],
                        cvec[0:1, P * c : P * (c + 1)].bitcast(f32),
                        cc[0:1, 1:2].bitcast(f32),
                        start=True,
                        stop=True,
                    )
                ccols = sm_pool.tile([P, NCH], f32, tag="ccols")
                nc.vector.tensor_copy(ccols[:], ps[:, 4:8].bitcast(f32))
                Rb = pr_pool.tile([P, D], f32, tag="Rb")
                nc.tensor.matmul(
                    Rb[:], onesr[:], cvec[:], start=True, stop=True
                )

                # ---- final centering on upper-triangle blocks + store ----
                ob = uo_pool.tile([P, OW], f32, tag="ob")
                for r in range(NCH):
                    w = CH_W[r]
                    cs = slice(P * r, D)
                    u = uo_pool.tile([P, w], f32, tag="u")
                    nc.vector.tensor_sub(u[:], dcs[r][:, cs].bitcast(f32), Rb[:, cs])
                    nc.vector.tensor_scalar_sub(
                        ob[:, OO[r] : OO[r] + w], u[:], ccols[:, r : r + 1]
                    )
                nc.sync.dma_start(out_d[s, 0:P, :], ob[:])

    # This walrus build accepts at most ONE sync wait per instruction.
    # Tile may attach several; hoist each extra wait onto its own no-op
    # placed just before the instruction (same engine, so ordering holds).
    if fixup:
        import bass_rust as _br

        for f in nc.m.functions:
            for blk in f.blocks:
                out_list = []
                changed = False
                for ins in blk.instructions:
                    si = getattr(ins, "sync_info", None)
                    if (
                        type(ins).__name__ != "InstNoOp"
                        and si is not None
                        and si.on_wait
                        and len(si.on_wait) > 1
                        and getattr(ins, "engine", None) is not None
                    ):
                        for j, w in enumerate(si.on_wait[:-1]):
                            nop = _br.InstNoOp(
                                name=f"I-w{j}-{ins.name}",
                                engine=ins.engine,
                                ins=[],
                                outs=[],
                            )
                            nop.sync_info = mybir.SyncInfo(
                                on_wait=[w], on_update=[]
                            )
                            out_list.append(nop)
                        ins.sync_info = mybir.SyncInfo(
                            on_wait=[si.on_wait[-1]], on_update=list(si.on_update)
                        )
                        changed = True
                    out_list.append(ins)
                if changed:
                    blk.instructions = out_list
    return nc


def make_consts(t_np):
    """Host-side constant tensors + the diagonal offset delta."""
    et = np.float32(np.exp(np.float32(t_np.reshape(-1)[0])))
    idc = np.zeros((P, 2 * P), dtype=np.float32)
    idc[:, 0:P] = np.eye(P)
    idc[:, P : 2 * P] = -GAMMA * np.eye(P)
    # diagonal argument of sqrt: 2*gamma*e^t + eps  (dpre_ii ~ 0)
    cval = np.float32(2.0 * GAMMA * et + EPS)
    delta = np.float32(np.sqrt(cval) - np.sqrt(np.float32(EPS)))
    cc = np.zeros((P, 4), dtype=np.float32)
    cc[:, 0] = -0.5
    cc[:, 1] = 1.0
    cc[:, 2] = 1.0 / (2.0 * D * D)
    cc[:, 3] = delta / (2.0 * D)  # delta/1024
    onesr = np.ones((1, P), dtype=np.float32)
    import ml_dtypes

    return {
        "idc": idc.astype(ml_dtypes.bfloat16),
        "cc": cc,
        "onesr": onesr,
    }, float(delta)


# triu assembly indices (static)
_TRIU_ROWSTART = np.zeros(D + 1, dtype=np.int64)
for _i in range(D):
    _TRIU_ROWSTART[_i + 1] = _TRIU_ROWSTART[_i] + (D - _i)
TRIU_LEN = int(_TRIU_ROWSTART[D])  # 131328


def assemble_triu(dev_out, delta):
    """dev_out: [n, P, OW] row-interleaved rectangles -> [n, 131328] triu."""
    n = dev_out.shape[0]
    out = np.empty((n, TRIU_LEN), dtype=np.float32)
    for r in range(NCH):
        for p in range(P):
            i = P * r + p
            s = _TRIU_ROWSTART[i]
            ln = D - i
            out[:, s : s + ln] = dev_out[:, p, OO[r] + p : OO[r] + p + ln]
            # fix the gamma-shifted diagonal entry
            out[:, s] -= delta
    return out


def make_in_maps(x, t):
    consts, delta = make_consts(t)
    in_maps = []
    for c in range(NCORES):
        m = {
            "x": np.ascontiguousarray(x[c * S : (c + 1) * S], dtype=np.float32),
            "t": np.asarray(t, dtype=np.float32).reshape(1, 1).copy(),
            "t128": np.broadcast_to(
                np.asarray(t, dtype=np.float32).reshape(1, 1), (P, 1)
            ).copy(),
        }
        m.update(consts)
        in_maps.append(m)
    return in_maps, delta


_CACHE = {}


def kernel(**inputs):
    import concourse.bass_utils as bass_utils

    x = np.ascontiguousarray(inputs["x"], dtype=np.float32)
    t = np.asarray(inputs["t"], dtype=np.float32)
    assert x.shape == (B, M, D)

    if "nc" not in _CACHE:
        _CACHE["nc"] = build_nc(S)
    nc = _CACHE["nc"]

    in_maps, delta = make_in_maps(x, t)

    res = bass_utils.run_bass_kernel_spmd(nc, in_maps, core_ids=list(range(NCORES)))
    full = np.empty((B, TRIU_LEN), dtype=np.float32)
    for c in range(NCORES):
        full[c * S : (c + 1) * S] = assemble_triu(res.results[c]["out"], delta)
    return full



# revision 12
# speedup vs baseline: 3.1748x; 3.1748x over previous
"""BDCovpool + Triuvec kernel for Trainium2 (8 NeuronCores, data-parallel).

Math (per sample b, x[b]: [M=196, D=512], t: scalar):
  gram[i,j] = sum_m x[m,i] x[m,j]           (D x D)
  d[i]      = gram[i,i]
  dpre      = d[i] + d[j] - 2 gram
  dcov      = sqrt(exp(t) * relu(dpre) + 1e-5)
  cent      = dcov - rowmean - colmean + totmean   (dcov symmetric -> row==col)
  out       = upper triangle of cent, row-major (131328 per sample)

Device strategy per core (32 samples, processed in 16 pairs):
  The tensor engine runs ONLY the gram stream (12 bf16 matmuls / sample,
  zero cross-engine deps), densely pipelined:
    PSUM[128,2048] = gram_chunks - (d'_i + d'_j)/2 - gamma*I_blockdiag
  The affine d' correction rides INSIDE the K=70 matmul via two host-packed
  augmentation rows on asymmetric lhsT/rhs tiles:
    xbl = [x(68) ; ones ; negh],  xbr = [x(68) ; negh ; ones],
  negh = -(d-196)/2 computed on host (0.4% of FLOPs). gamma=256 keeps the
  noisy ~0 diagonal sqrt argument positive.
  dcov = ONE ACT Sqrt(PSUM * (-2 e^t) + (392 e^t + eps)) -> f32 [128,2048]
  (constant bias, since both d' halves are in PSUM). bf16 dcov storage would
  inject ~2e-3 abs noise, fatal after centering where the signal is ~0.04.
  Row sums on DVE (one [128,4,512] X-reduce -> rsum[128,4]), then the
  centering vector c = rowmean - tot/2 in column form (3 tiny DVE ops).
Outputs: upper-triangle rectangles of dcov (f32, 4 DMAs issued from the
idle gpsimd queue) + c columns. Host applies cent = dcov - c_i - c_j while
unpacking rows and fixes the gamma-shifted diagonal.
"""

import numpy as np

B, M, D = 256, 196, 512
NCORES = 8
S = B // NCORES  # samples per core
NPAIR = S // 2  # 16 sample-pairs per core
P = 128
NCH = D // P  # 4 row chunks
MB = M - P  # 68 rows in second k-chunk
MA = MB + 2  # 70 = augmented k-chunk (x rows + 2 correction rows)
GAMMA = 256.0  # diagonal shift (bf16-exact); sqrt argument ~2*gamma*e^t ~ 1.3
EPS = 1e-5
KC = float(M)
# per-sample output rectangles [128, 512-128r], row-interleaved
CH_W = [D - P * r for r in range(NCH)]
OO = [0]
for r in range(NCH - 1):
    OO.append(OO[-1] + CH_W[r])
OW = OO[-1] + CH_W[-1]  # 1280


def build_nc(n_samples=S, fixup=True):
    import concourse.bass as bass
    import concourse.mybir as mybir
    import concourse.tile as tile

    f32 = mybir.dt.float32
    bf16 = mybir.dt.bfloat16
    AF = mybir.ActivationFunctionType
    ALU = mybir.AluOpType

    npair = n_samples // 2

    nc = bass.Bass(
        "TRN2",
        target_bir_lowering=False,
        debug=False,
        enable_asserts=False,
    )

    # pair-packed inputs: xal[p, :, k*512:(k+1)*512] = x[2p+k, 0:128, :]
    xal_d = nc.dram_tensor("xal", [npair, P, 2 * D], bf16, kind="ExternalInput").ap()
    xbl_d = nc.dram_tensor("xbl", [npair, MA, 2 * D], bf16, kind="ExternalInput").ap()
    xbr_d = nc.dram_tensor("xbr", [npair, MA, 2 * D], bf16, kind="ExternalInput").ap()
    t128_d = nc.dram_tensor("t128", [P, 1], f32, kind="ExternalInput").ap()
    idc_d = nc.dram_tensor("idc", [P, 2 * P], bf16, kind="ExternalInput").ap()
    rect_d = nc.dram_tensor(
        "rect", [npair, 2, P, OW], f32, kind="ExternalOutput"
    ).ap()
    # raw row sums (column form); the tiny scalar combine happens on host
    ccf_d = nc.dram_tensor("ccf", [npair, P, 2 * NCH], f32, kind="ExternalOutput").ap()

    with tile.TileContext(nc) as tc:
        with (
            tc.tile_pool(name="const", bufs=1) as cpool,
            tc.tile_pool(name="xa", bufs=3) as xa_pool,
            tc.tile_pool(name="xb", bufs=3) as xb_pool,
            tc.tile_pool(name="dcov", bufs=3) as dc_pool,
            tc.tile_pool(name="ccfp", bufs=3) as cc_pool,
            tc.tile_pool(name="pg", bufs=2, space="PSUM") as pg_pool,
        ):
            # ---- once-per-kernel setup ----
            idc = cpool.tile([P, 2 * P], bf16, tag="idc")
            nc.sync.dma_start(idc[:], idc_d[:])
            t128 = cpool.tile([P, 1], f32, tag="t128")
            nc.sync.dma_start(t128[:], t128_d[:])
            et128 = cpool.tile([P, 1], f32, tag="et128")
            nc.scalar.activation(et128[:], t128[:], AF.Exp)
            scb = cpool.tile([P, 2], f32, tag="scb")
            # scb col0 = -2 e^t (sqrt scale); col1 = 392 e^t + eps (bias const)
            nc.vector.tensor_scalar_mul(scb[:, 0:1], et128[:], -2.0)
            nc.vector.tensor_scalar(
                scb[:, 1:2], et128[:], 2.0 * KC, EPS, ALU.mult, ALU.add
            )
            sc_ap = scb[:, 0:1]
            bias_ap = scb[:, 1:2]

            for pr in range(npair):
                # ---- load x pair (gram inputs only; no other PE deps) ----
                xa = xa_pool.tile([P, 2 * D], bf16, tag="xa")
                nc.sync.dma_start(xa[:], xal_d[pr, :, :])
                xbl = xb_pool.tile([MA, 2 * D], bf16, tag="xbl")
                nc.sync.dma_start(xbl[:], xbl_d[pr, :, :])
                xbr = xb_pool.tile([MA, 2 * D], bf16, tag="xbr")
                nc.sync.dma_start(xbr[:], xbr_d[pr, :, :])

                ccf = cc_pool.tile([P, 2 * NCH], f32, tag="ccf")
                for k in range(2):
                    ks = slice(k * D, (k + 1) * D)

                    # ---- gram + fused affine/gamma corrections ----
                    pg = pg_pool.tile([P, NCH, D], f32, tag="pg")
                    for r in range(NCH):
                        sl = slice(k * D + P * r, k * D + P * (r + 1))
                        nc.tensor.matmul(
                            pg[:, r, :], xa[:, sl], xa[:, ks],
                            start=True, stop=False,
                        )
                        nc.tensor.matmul(
                            pg[:, r, :], xbl[:, sl], xbr[:, ks],
                            start=False, stop=False,
                        )
                        # - gamma I on the diagonal block
                        nc.tensor.matmul(
                            pg[:, r, P * r : P * (r + 1)],
                            idc[:, 0:P],
                            idc[:, P : 2 * P],
                            start=False,
                            stop=True,
                        )

                    # ---- dcov = sqrt(pg * (-2e^t) + (392e^t + eps)) ----
                    dc = dc_pool.tile([P, NCH, D], f32, tag="dcov")
                    nc.scalar.activation(
                        dc[:, :, :], pg[:, :, :], AF.Sqrt, bias=bias_ap, scale=sc_ap
                    )

                    # ---- row sums (DVE, column form); combine on host ----
                    nc.vector.tensor_reduce(
                        ccf[:, k * NCH : (k + 1) * NCH],
                        dc[:, :, :],
                        mybir.AxisListType.X,
                        ALU.add,
                    )

                    # ---- ship upper-tri rectangles (idle gpsimd queue) ----
                    for r in range(NCH):
                        w = CH_W[r]
                        nc.gpsimd.dma_start(
                            rect_d[pr, k, :, OO[r] : OO[r] + w],
                            dc[:, r, P * r : D],
                        )
                nc.sync.dma_start(ccf_d[pr, :, :], ccf[:])

    # This walrus build accepts at most ONE sync wait per instruction.
    # Tile may attach several; hoist each extra wait onto its own no-op
    # placed just before the instruction (same engine, so ordering holds).
    if fixup:
        import bass_rust as _br

        for f in nc.m.functions:
            for blk in f.blocks:
                out_list = []
                changed = False
                for ins in blk.instructions:
                    si = getattr(ins, "sync_info", None)
                    if (
                        type(ins).__name__ != "InstNoOp"
                        and si is not None
                        and si.on_wait
                        and len(si.on_wait) > 1
                        and getattr(ins, "engine", None) is not None
                    ):
                        for j, w in enumerate(si.on_wait[:-1]):
                            nop = _br.InstNoOp(
                                name=f"I-w{j}-{ins.name}",
                                engine=ins.engine,
                                ins=[],
                                outs=[],
                            )
                            nop.sync_info = mybir.SyncInfo(
                                on_wait=[w], on_update=[]
                            )
                            out_list.append(nop)
                        ins.sync_info = mybir.SyncInfo(
                            on_wait=[si.on_wait[-1]], on_update=list(si.on_update)
                        )
                        changed = True
                    out_list.append(ins)
                if changed:
                    blk.instructions = out_list
    return nc


def make_consts(t_np):
    """Host-side constant tensors + the diagonal offset delta."""
    import ml_dtypes

    bf = ml_dtypes.bfloat16
    et = np.float32(np.exp(np.float32(np.asarray(t_np).reshape(-1)[0])))
    idc = np.zeros((P, 2 * P), dtype=np.float32)
    idc[:, 0:P] = np.eye(P)
    idc[:, P : 2 * P] = -GAMMA * np.eye(P)
    # diagonal argument of sqrt: 2*gamma*e^t + eps  (dpre_ii ~ 0)
    cval = np.float32(2.0 * GAMMA * et + EPS)
    delta = np.float32(np.sqrt(cval) - np.sqrt(np.float32(EPS)))
    return {"idc": idc.astype(bf)}, float(delta)


# triu assembly indices (static)
_TRIU_ROWSTART = np.zeros(D + 1, dtype=np.int64)
for _i in range(D):
    _TRIU_ROWSTART[_i + 1] = _TRIU_ROWSTART[_i] + (D - _i)
TRIU_LEN = int(_TRIU_ROWSTART[D])  # 131328


def pack_x(xc):
    """xc: [n, M, D] f32 -> bf16 pair-packed (xal, xbl, xbr)."""
    import ml_dtypes

    bf = ml_dtypes.bfloat16
    n = xc.shape[0]
    d = np.einsum("smd,smd->sd", xc, xc, dtype=np.float32)
    negh = (-0.5 * (d - KC)).astype(np.float32)
    xb16 = xc.astype(bf)

    def pairify(a):  # [n, R, D] -> [n/2, R, 2D]
        return np.ascontiguousarray(
            a.reshape(n // 2, 2, a.shape[1], D).transpose(0, 2, 1, 3)
        ).reshape(n // 2, a.shape[1], 2 * D)

    xal = pairify(xb16[:, 0:P, :])
    aug_l = np.empty((n, 2, D), dtype=bf)
    aug_l[:, 0, :] = 1.0
    aug_l[:, 1, :] = negh.astype(bf)
    aug_r = np.empty((n, 2, D), dtype=bf)
    aug_r[:, 0, :] = negh.astype(bf)
    aug_r[:, 1, :] = 1.0
    xmid = xb16[:, P:M, :]
    xbl = pairify(np.concatenate([xmid, aug_l], axis=1))
    xbr = pairify(np.concatenate([xmid, aug_r], axis=1))
    return xal, xbl, xbr


def assemble(rect, ccf, delta):
    """rect: [npair,2,P,OW] f32, ccf: [npair,P,8] raw rowsums -> cent."""
    npair = rect.shape[0]
    n = npair * 2
    d4 = rect.reshape(n, P, OW)
    # rs[s, 128r+p] = ccf[pair, p, 4k+r]  (raw rowsums, gamma'd diagonal)
    rs = (
        ccf.reshape(npair, P, 2, NCH)
        .transpose(0, 2, 3, 1)
        .reshape(n, D)
        .astype(np.float64)
    )
    rs -= delta  # remove the gamma-shifted diagonal contribution
    tot = rs.sum(axis=1, keepdims=True) / (D * D)
    c = (rs / D - tot / 2).astype(np.float32)
    out = np.empty((n, TRIU_LEN), dtype=np.float32)
    for r in range(NCH):
        for p in range(P):
            i = P * r + p
            s = _TRIU_ROWSTART[i]
            ln = D - i
            out[:, s : s + ln] = (
                d4[:, p, OO[r] + p : OO[r] + p + ln]
                - c[:, i : i + 1]
                - c[:, i:D]
            )
            # fix the gamma-shifted diagonal entry
            out[:, s] -= delta
    return out


def make_in_maps(x, t):
    consts, delta = make_consts(t)
    t128 = np.broadcast_to(
        np.asarray(t, dtype=np.float32).reshape(1, 1), (P, 1)
    ).copy()
    in_maps = []
    for c in range(NCORES):
        xal, xbl, xbr = pack_x(np.asarray(x[c * S : (c + 1) * S], dtype=np.float32))
        m = {"xal": xal, "xbl": xbl, "xbr": xbr, "t128": t128}
        m.update(consts)
        in_maps.append(m)
    return in_maps, delta


_CACHE = {}


def kernel(**inputs):
    import concourse.bass_utils as bass_utils

    x = np.ascontiguousarray(inputs["x"], dtype=np.float32)
    t = np.asarray(inputs["t"], dtype=np.float32)
    assert x.shape == (B, M, D)

    if "nc" not in _CACHE:
        _CACHE["nc"] = build_nc(S)
    nc = _CACHE["nc"]

    in_maps, delta = make_in_maps(x, t)

    res = bass_utils.run_bass_kernel_spmd(nc, in_maps, core_ids=list(range(NCORES)))
    full = np.empty((B, TRIU_LEN), dtype=np.float32)
    for c in range(NCORES):
        full[c * S : (c + 1) * S] = assemble(
            res.results[c]["rect"], res.results[c]["ccf"], delta
        )
    return full


# revision 15
# speedup vs baseline: 4.3334x; 1.3649x over previous
"""BDCovpool + Triuvec kernel for Trainium2 (8 NeuronCores, data-parallel).

Math (per sample b, x[b]: [M=196, D=512], t: scalar):
  gram[i,j] = sum_m x[m,i] x[m,j]           (D x D)
  d[i]      = gram[i,i]
  dpre      = d[i] + d[j] - 2 gram
  dcov      = sqrt(exp(t) * relu(dpre) + 1e-5)
  cent      = dcov - rowmean - colmean + totmean   (dcov symmetric -> row==col)
  out       = upper triangle of cent, row-major (131328 per sample)

Device strategy per core (32 samples, processed in 16 pairs):
  The tensor engine runs an upper-triangle-only gram stream with zero
  cross-engine deps: for row-block r, the moving rhs is sliced to columns
  >= 128r, so PSUM comes out PACKED as the output rectangles [128, 1280]:
    PSUM = gram_upper - (d'_i + d'_j)/2 - gamma*I_blockdiag
  The affine d' correction rides INSIDE the K=70 matmul via two host-packed
  augmentation rows on asymmetric lhsT/rhs tiles:
    xbl = [x(68) ; ones ; negh],  xbr = [x(68) ; negh ; ones],
  negh = -(d-196)/2 computed on host (0.4% of FLOPs). gamma=256 keeps the
  noisy ~0 diagonal sqrt argument positive.
  dcov = ONE ACT Sqrt(PSUM * (-2 e^t) + (392 e^t + eps)) -> fp16 [128,1280]
  (constant bias since both d' halves are in PSUM; fp16 keeps the ~1.0
  dcov values to ~3.5e-4 abs, vs 2e-3 for bf16 which would fail after
  centering where the signal rms is 0.074).
  Row sums: upper-rect sums on DVE (4 X-reduces) + the missing below-diag
  parts from 6 tiny transposed-colsum matmuls (lhsT = dcov slice, rhs =
  ones column, ap=1) accumulated in PSUM; combined on DVE into ccf.
Outputs: the fp16 dcov rectangles (ONE DMA per sample, gpsimd queue) +
row-sum columns (one DMA at the end). Host applies the double centering
cent = dcov - c_i - c_j while unpacking rows, and fixes the gamma diagonal.
"""

import numpy as np

B, M, D = 256, 196, 512
NCORES = 8
S = B // NCORES  # samples per core
NPAIR = S // 2  # 16 sample-pairs per core
P = 128
NCH = D // P  # 4 row chunks
MB = M - P  # 68 rows in second k-chunk
MA = MB + 2  # 70 = augmented k-chunk (x rows + 2 correction rows)
GAMMA = 256.0  # diagonal shift; sqrt argument ~2*gamma*e^t ~ 1.3
EPS = 1e-5
KC = float(M)
# packed upper-triangle rectangles [128, 512-128r]
CH_W = [D - P * r for r in range(NCH)]
OO = [0]
for r in range(NCH - 1):
    OO.append(OO[-1] + CH_W[r])
OW = OO[-1] + CH_W[-1]  # 1280


def build_nc(n_samples=S, fixup=True):
    import concourse.bass as bass
    import concourse.mybir as mybir
    import concourse.tile as tile

    f32 = mybir.dt.float32
    f16 = mybir.dt.float16
    bf16 = mybir.dt.bfloat16
    AF = mybir.ActivationFunctionType
    ALU = mybir.AluOpType

    npair = n_samples // 2

    nc = bass.Bass(
        "TRN2",
        target_bir_lowering=False,
        debug=False,
        enable_asserts=False,
    )

    # pair-packed inputs: xal[p, :, k*512:(k+1)*512] = x[2p+k, 0:128, :]
    xal_d = nc.dram_tensor("xal", [npair, P, 2 * D], bf16, kind="ExternalInput").ap()
    # xblr = [xbl-pair | xbr-pair] along free dim (one DMA)
    xblr_d = nc.dram_tensor(
        "xblr", [npair, MA, 4 * D], bf16, kind="ExternalInput"
    ).ap()
    t128_d = nc.dram_tensor("t128", [P, 1], f32, kind="ExternalInput").ap()
    idc_d = nc.dram_tensor("idc", [P, 2 * P], bf16, kind="ExternalInput").ap()
    onesh_d = nc.dram_tensor("onesh", [P, 1], f16, kind="ExternalInput").ap()
    rect_d = nc.dram_tensor(
        "rect", [npair, 2, P, OW], f16, kind="ExternalOutput"
    ).ap()
    # raw row sums (column form); the tiny scalar combine happens on host
    ccf_d = nc.dram_tensor("ccf", [P, 8 * npair], f32, kind="ExternalOutput").ap()

    PGW = 3 * D  # pg padded to 3 PSUM banks (1536 cols; 1280 used)

    with tile.TileContext(nc) as tc:
        with (
            tc.tile_pool(name="const", bufs=1) as cpool,
            tc.tile_pool(name="xa", bufs=3) as xa_pool,
            tc.tile_pool(name="xb", bufs=3) as xb_pool,
            tc.tile_pool(name="dcov", bufs=3) as dc_pool,
            tc.tile_pool(name="pg", bufs=2, space="PSUM") as pg_pool,
            tc.tile_pool(name="pcol", bufs=2, space="PSUM") as pc_pool,
        ):
            # ---- once-per-kernel setup ----
            idc = cpool.tile([P, 2 * P], bf16, tag="idc")
            nc.sync.dma_start(idc[:], idc_d[:])
            onesh = cpool.tile([P, 1], f16, tag="onesh")
            nc.sync.dma_start(onesh[:], onesh_d[:])
            t128 = cpool.tile([P, 1], f32, tag="t128")
            nc.sync.dma_start(t128[:], t128_d[:])
            et128 = cpool.tile([P, 1], f32, tag="et128")
            nc.scalar.activation(et128[:], t128[:], AF.Exp)
            scb = cpool.tile([P, 2], f32, tag="scb")
            # scb col0 = -2 e^t (sqrt scale); col1 = 392 e^t + eps (bias const)
            nc.vector.tensor_scalar_mul(scb[:, 0:1], et128[:], -2.0)
            nc.vector.tensor_scalar(
                scb[:, 1:2], et128[:], 2.0 * KC, EPS, ALU.mult, ALU.add
            )
            sc_ap = scb[:, 0:1]
            bias_ap = scb[:, 1:2]
            ccf = cpool.tile([P, 8 * npair], f32, tag="ccf")

            prev = None  # deferred per-sample tail: (dc, pr, k)
            for pr in range(npair):
                # ---- load x pair (gram inputs only; no other PE deps) ----
                xa = xa_pool.tile([P, 2 * D], bf16, tag="xa")
                nc.sync.dma_start(xa[:], xal_d[pr, :, :])
                xblr = xb_pool.tile([MA, 4 * D], bf16, tag="xblr")
                nc.sync.dma_start(xblr[:], xblr_d[pr, :, :])

                for k in range(2):
                    # ---- upper-packed gram + fused corrections ----
                    # PSUM rects bank-aligned (matmul out can't cross banks):
                    # cols [0:512][512:896][1024:1280][1280:1408]
                    pg = pg_pool.tile([P, PGW], f32, tag="pg")
                    PQ = [0, 512, 1024, 1280]
                    for r in range(NCH):
                        q = PQ[r]
                        w = CH_W[r]
                        lsl = slice(k * D + P * r, k * D + P * (r + 1))
                        usl = slice(k * D + P * r, (k + 1) * D)
                        nc.tensor.matmul(
                            pg[:, q : q + w], xa[:, lsl], xa[:, usl],
                            start=True, stop=False,
                        )
                        nc.tensor.matmul(
                            pg[:, q : q + w],
                            xblr[:, lsl],
                            xblr[:, 2 * D + P * r + k * D : 2 * D + (k + 1) * D],
                            start=False, stop=False,
                        )
                        # - gamma I on the (leading) diagonal block
                        nc.tensor.matmul(
                            pg[:, q : q + P],
                            idc[:, 0:P],
                            idc[:, P : 2 * P],
                            start=False,
                            stop=True,
                        )

                    # ---- deferred tail of the PREVIOUS sample (keeps the
                    # tensor queue from stalling on this sample's ACT) ----
                    if prev is not None:
                        self_tail(nc, mybir, prev, onesh, ccf, pc_pool, rect_d)
                    # ---- dcov = sqrt(pg*(-2e^t) + (392e^t+eps)) -> fp16,
                    # two ACTs around the PSUM alignment gap; dc ends up
                    # tightly packed [128, 1280] in rect layout ----
                    dc = dc_pool.tile([P, OW], f16, tag="dcov")
                    nc.scalar.activation(
                        dc[:, 0:896], pg[:, 0:896], AF.Sqrt,
                        bias=bias_ap, scale=sc_ap,
                    )
                    nc.scalar.activation(
                        dc[:, 896:OW], pg[:, 1024 : 1024 + OW - 896], AF.Sqrt,
                        bias=bias_ap, scale=sc_ap,
                    )
                    prev = (dc, pr, k)
            self_tail(nc, mybir, prev, onesh, ccf, pc_pool, rect_d)
            nc.sync.dma_start(ccf_d[:, :], ccf[:])

    # This walrus build accepts at most ONE sync wait per instruction.
    # Tile may attach several; hoist each extra wait onto its own no-op
    # placed just before the instruction (same engine, so ordering holds).
    if fixup:
        import bass_rust as _br

        for f in nc.m.functions:
            for blk in f.blocks:
                out_list = []
                changed = False
                for ins in blk.instructions:
                    si = getattr(ins, "sync_info", None)
                    if (
                        type(ins).__name__ != "InstNoOp"
                        and si is not None
                        and si.on_wait
                        and len(si.on_wait) > 1
                        and getattr(ins, "engine", None) is not None
                    ):
                        for j, w in enumerate(si.on_wait[:-1]):
                            nop = _br.InstNoOp(
                                name=f"I-w{j}-{ins.name}",
                                engine=ins.engine,
                                ins=[],
                                outs=[],
                            )
                            nop.sync_info = mybir.SyncInfo(
                                on_wait=[w], on_update=[]
                            )
                            out_list.append(nop)
                        ins.sync_info = mybir.SyncInfo(
                            on_wait=[si.on_wait[-1]], on_update=list(si.on_update)
                        )
                        changed = True
                    out_list.append(ins)
                if changed:
                    blk.instructions = out_list
    return nc


def self_tail(nc, mybir, prev, onesh, ccf, pc_pool, rect_d):
    """Row sums + rect DMA for a finished sample (dc ready)."""
    ALU = mybir.AluOpType
    f32 = mybir.dt.float32
    dc, pr, k = prev
    cc0 = 8 * pr + 4 * k
    # below-diagonal rowsum parts: transposed colsums, ap=1 matmuls
    pcol = pc_pool.tile([P, NCH], f32, tag="pcol")
    for r in range(1, NCH):
        for rp in range(r):
            seg = OO[rp] + P * (r - rp)
            nc.tensor.matmul(
                pcol[:, r : r + 1],
                dc[:, seg : seg + P],
                onesh[:],
                start=(rp == 0),
                stop=(rp == r - 1),
            )
    # upper-rect row sums (fp16 input -> 2x DVE mode)
    for r in range(NCH):
        nc.vector.tensor_reduce(
            ccf[:, cc0 + r : cc0 + r + 1],
            dc[:, OO[r] : OO[r] + CH_W[r]],
            mybir.AxisListType.X,
            ALU.add,
        )
    # add the below-diagonal parts for r>=1
    nc.vector.tensor_tensor(
        ccf[:, cc0 + 1 : cc0 + 4],
        ccf[:, cc0 + 1 : cc0 + 4],
        pcol[:, 1:4],
        ALU.add,
    )
    # ship the packed rectangles (idle gpsimd queue)
    nc.gpsimd.dma_start(rect_d[pr, k, :, :], dc[:])


def make_consts(t_np):
    """Host-side constant tensors + the diagonal offset delta."""
    import ml_dtypes

    bf = ml_dtypes.bfloat16
    et = np.float32(np.exp(np.float32(np.asarray(t_np).reshape(-1)[0])))
    idc = np.zeros((P, 2 * P), dtype=np.float32)
    idc[:, 0:P] = np.eye(P)
    idc[:, P : 2 * P] = -GAMMA * np.eye(P)
    # diagonal argument of sqrt: 2*gamma*e^t + eps  (dpre_ii ~ 0)
    cval = np.float32(2.0 * GAMMA * et + EPS)
    delta = np.float32(np.sqrt(cval) - np.sqrt(np.float32(EPS)))
    return {
        "idc": idc.astype(bf),
        "onesh": np.ones((P, 1), dtype=np.float16),
    }, float(delta)


# triu assembly indices (static)
_TRIU_ROWSTART = np.zeros(D + 1, dtype=np.int64)
for _i in range(D):
    _TRIU_ROWSTART[_i + 1] = _TRIU_ROWSTART[_i] + (D - _i)
TRIU_LEN = int(_TRIU_ROWSTART[D])  # 131328


def pack_x(xc):
    """xc: [n, M, D] f32 -> bf16 pair-packed (xal, xblr)."""
    import ml_dtypes

    bf = ml_dtypes.bfloat16
    n = xc.shape[0]
    d = np.einsum("smd,smd->sd", xc, xc, dtype=np.float32)
    negh = (-0.5 * (d - KC)).astype(np.float32)
    xb16 = xc.astype(bf)

    def pairify(a):  # [n, R, D] -> [n/2, R, 2D]
        return np.ascontiguousarray(
            a.reshape(n // 2, 2, a.shape[1], D).transpose(0, 2, 1, 3)
        ).reshape(n // 2, a.shape[1], 2 * D)

    xal = pairify(xb16[:, 0:P, :])
    aug_l = np.empty((n, 2, D), dtype=bf)
    aug_l[:, 0, :] = 1.0
    aug_l[:, 1, :] = negh.astype(bf)
    aug_r = np.empty((n, 2, D), dtype=bf)
    aug_r[:, 0, :] = negh.astype(bf)
    aug_r[:, 1, :] = 1.0
    xmid = xb16[:, P:M, :]
    xbl = pairify(np.concatenate([xmid, aug_l], axis=1))
    xbr = pairify(np.concatenate([xmid, aug_r], axis=1))
    xblr = np.ascontiguousarray(np.concatenate([xbl, xbr], axis=2))
    return xal, xblr


def assemble(rect, ccf, delta):
    """rect: [npair,2,P,OW] f16, ccf: [P, 8*npair] raw rowsums -> cent."""
    npair = rect.shape[0]
    n = npair * 2
    d4 = rect.astype(np.float32).reshape(n, P, OW)
    # rs[s, 128r+p] = ccf[p, 8*pr+4*k+r]  (raw rowsums, gamma'd diagonal)
    rs = (
        ccf.reshape(P, npair, 2, NCH)
        .transpose(1, 2, 3, 0)
        .reshape(n, D)
        .astype(np.float64)
    )
    rs -= delta  # remove the gamma-shifted diagonal contribution
    tot = rs.sum(axis=1, keepdims=True) / (D * D)
    c = (rs / D - tot / 2).astype(np.float32)
    out = np.empty((n, TRIU_LEN), dtype=np.float32)
    for r in range(NCH):
        for p in range(P):
            i = P * r + p
            s = _TRIU_ROWSTART[i]
            ln = D - i
            out[:, s : s + ln] = (
                d4[:, p, OO[r] + p : OO[r] + p + ln]
                - c[:, i : i + 1]
                - c[:, i:D]
            )
            # fix the gamma-shifted diagonal entry
            out[:, s] -= delta
    return out


def make_in_maps(x, t):
    consts, delta = make_consts(t)
    t128 = np.broadcast_to(
        np.asarray(t, dtype=np.float32).reshape(1, 1), (P, 1)
    ).copy()
    in_maps = []
    for c in range(NCORES):
        xal, xblr = pack_x(np.asarray(x[c * S : (c + 1) * S], dtype=np.float32))
        m = {"xal": xal, "xblr": xblr, "t128": t128}
        m.update(consts)
        in_maps.append(m)
    return in_maps, delta


_CACHE = {}


def kernel(**inputs):
    import concourse.bass_utils as bass_utils

    x = np.ascontiguousarray(inputs["x"], dtype=np.float32)
    t = np.asarray(inputs["t"], dtype=np.float32)
    assert x.shape == (B, M, D)

    if "nc" not in _CACHE:
        _CACHE["nc"] = build_nc(S)
    nc = _CACHE["nc"]

    in_maps, delta = make_in_maps(x, t)

    res = bass_utils.run_bass_kernel_spmd(nc, in_maps, core_ids=list(range(NCORES)))
    full = np.empty((B, TRIU_LEN), dtype=np.float32)
    for c in range(NCORES):
        full[c * S : (c + 1) * S] = assemble(
            res.results[c]["rect"], res.results[c]["ccf"], delta
        )
    return full


# revision 17
# speedup vs baseline: 4.4305x; 1.0224x over previous
"""BDCovpool + Triuvec kernel for Trainium2 (8 NeuronCores, data-parallel).

Math (per sample b, x[b]: [M=196, D=512], t: scalar):
  gram[i,j] = sum_m x[m,i] x[m,j]           (D x D)
  d[i]      = gram[i,i]
  dpre      = d[i] + d[j] - 2 gram
  dcov      = sqrt(exp(t) * relu(dpre) + 1e-5)
  cent      = dcov - rowmean - colmean + totmean   (dcov symmetric -> row==col)
  out       = upper triangle of cent, row-major (131328 per sample)

Device strategy per core (32 samples, processed in 16 pairs):
  The tensor engine runs an upper-block gram stream with zero cross-engine
  deps. For row-block r the moving rhs is sliced to columns >= 128r (block 1
  keeps full width so the PSUM packs gap-free into 3 banks):
    PSUM[128,1408] = [blk0 512 | blk1 512 | blk2 256 | blk3 128]
                   = gram_blocks - (d'_i + d'_j)/2 - gamma*I_blockdiag
  The affine d' correction rides INSIDE the K=70 matmul via two host-packed
  augmentation rows on asymmetric lhsT/rhs tiles:
    xbl = [x(68) ; ones ; negh],  xbr = [x(68) ; negh ; ones],
  negh = -(d-196)/2 computed on host (0.4% of FLOPs). gamma=256 keeps the
  noisy ~0 diagonal sqrt argument positive.
  dcov = ONE ACT Sqrt(PSUM * (-2 e^t) + (392 e^t + eps)) -> fp16 [128,1408]
  (constant bias since both d' halves are in PSUM; fp16 keeps the ~1.0
  dcov values to ~3.5e-4 abs, vs 2e-3 for bf16 which would fail after
  centering where the signal rms is 0.074).
  Row sums: per-block sums on DVE (tensor_scalar+accum_out, fp16 2x) + the
  missing below-diag parts from 5 tiny transposed-colsum matmuls (lhsT =
  dcov slice, rhs = ones column, ap=1) accumulated in PSUM.
  Each sample's tail (colsums, row sums, output DMA) is deferred by TWO
  samples so no engine ever waits on this sample's ACT.
Outputs: the fp16 dcov blocks (ONE DMA per sample, gpsimd queue) + row-sum
columns (one DMA at the end). Host applies the double centering
cent = dcov - c_i - c_j while unpacking rows, and fixes the gamma diagonal.
"""

import numpy as np

B, M, D = 256, 196, 512
NCORES = 8
S = B // NCORES  # samples per core
NPAIR = S // 2  # 16 sample-pairs per core
P = 128
NCH = D // P  # 4 row chunks
MB = M - P  # 68 rows in second k-chunk
MA = MB + 2  # 70 = augmented k-chunk (x rows + 2 correction rows)
GAMMA = 256.0  # diagonal shift; sqrt argument ~2*gamma*e^t ~ 1.3
EPS = 1e-5
KC = float(M)
# device block layout: block r starts at DCQ[r], covers gram cols >= JB[r]
DCQ = [0, 512, 1024, 1280]
JB = [0, 0, 256, 384]
BW = [512, 512, 256, 128]  # block widths
OW = 1408


def build_nc(n_samples=S, fixup=True):
    import concourse.bass as bass
    import concourse.mybir as mybir
    import concourse.tile as tile

    f32 = mybir.dt.float32
    f16 = mybir.dt.float16
    bf16 = mybir.dt.bfloat16
    AF = mybir.ActivationFunctionType
    ALU = mybir.AluOpType

    npair = n_samples // 2

    nc = bass.Bass(
        "TRN2",
        target_bir_lowering=False,
        debug=False,
        enable_asserts=False,
    )

    # pair-packed inputs: xal[p, :, k*512:(k+1)*512] = x[2p+k, 0:128, :]
    xal_d = nc.dram_tensor("xal", [npair, P, 2 * D], bf16, kind="ExternalInput").ap()
    # xblr = [xbl-pair | xbr-pair] along free dim (one DMA)
    xblr_d = nc.dram_tensor(
        "xblr", [npair, MA, 4 * D], bf16, kind="ExternalInput"
    ).ap()
    t128_d = nc.dram_tensor("t128", [P, 1], f32, kind="ExternalInput").ap()
    idc_d = nc.dram_tensor("idc", [P, 2 * P], bf16, kind="ExternalInput").ap()
    onesh_d = nc.dram_tensor("onesh", [P, 1], f16, kind="ExternalInput").ap()
    rect_d = nc.dram_tensor(
        "rect", [npair, 2, P, OW], f16, kind="ExternalOutput"
    ).ap()
    # raw row sums (column form); the tiny scalar combine happens on host
    ccf_d = nc.dram_tensor("ccf", [P, 8 * npair], f32, kind="ExternalOutput").ap()

    PGW = 3 * D  # pg = 3 PSUM banks (1536 cols; 1408 used)

    state = {}

    def tail(prev):
        """Row sums + rect DMA for a finished sample (dc long ready)."""
        dc, pr, k = prev
        cc0 = 8 * pr + 4 * k
        # below-diagonal rowsum parts for blocks 2,3: transposed colsums
        pcol = state["pc_pool"].tile([P, NCH], f32, tag="pcol")
        for r in range(2, NCH):
            for rp in range(r):
                seg = DCQ[rp] + P * r - JB[rp]
                nc.tensor.matmul(
                    pcol[:, r : r + 1],
                    dc[:, seg : seg + P],
                    state["onesh"][:],
                    start=(rp == 0),
                    stop=(rp == r - 1),
                )
        # per-block row sums (fp16 tensor_scalar + accum -> 2x DVE mode)
        ccf = state["ccf"]
        scr = state["scr_pool"].tile([P, D], f16, tag="scr")
        for r in range(NCH):
            nc.vector.tensor_scalar(
                scr[:, 0 : BW[r]],
                dc[:, DCQ[r] : DCQ[r] + BW[r]],
                1.0,
                0.0,
                ALU.mult,
                ALU.add,
                accum_out=ccf[:, cc0 + r : cc0 + r + 1],
            )
        # add the below-diagonal parts for r=2,3
        nc.vector.tensor_tensor(
            ccf[:, cc0 + 2 : cc0 + 4],
            ccf[:, cc0 + 2 : cc0 + 4],
            pcol[:, 2:4],
            ALU.add,
        )
        # ship the packed blocks (idle gpsimd queue)
        nc.gpsimd.dma_start(rect_d[pr, k, :, :], dc[:])

    with tile.TileContext(nc) as tc:
        with (
            tc.tile_pool(name="const", bufs=1) as cpool,
            tc.tile_pool(name="xa", bufs=3) as xa_pool,
            tc.tile_pool(name="xb", bufs=3) as xb_pool,
            tc.tile_pool(name="dcov", bufs=4) as dc_pool,
            tc.tile_pool(name="scr", bufs=2) as scr_pool,
            tc.tile_pool(name="pg", bufs=2, space="PSUM") as pg_pool,
            tc.tile_pool(name="pcol", bufs=2, space="PSUM") as pc_pool,
        ):
            # ---- once-per-kernel setup ----
            idc = cpool.tile([P, 2 * P], bf16, tag="idc")
            nc.sync.dma_start(idc[:], idc_d[:])
            onesh = cpool.tile([P, 1], f16, tag="onesh")
            nc.sync.dma_start(onesh[:], onesh_d[:])
            t128 = cpool.tile([P, 1], f32, tag="t128")
            nc.sync.dma_start(t128[:], t128_d[:])
            et128 = cpool.tile([P, 1], f32, tag="et128")
            nc.scalar.activation(et128[:], t128[:], AF.Exp)
            scb = cpool.tile([P, 2], f32, tag="scb")
            # scb col0 = -2 e^t (sqrt scale); col1 = 392 e^t + eps (bias const)
            nc.vector.tensor_scalar_mul(scb[:, 0:1], et128[:], -2.0)
            nc.vector.tensor_scalar(
                scb[:, 1:2], et128[:], 2.0 * KC, EPS, ALU.mult, ALU.add
            )
            sc_ap = scb[:, 0:1]
            bias_ap = scb[:, 1:2]
            ccf = cpool.tile([P, 8 * npair], f32, tag="ccf")
            state.update(
                ccf=ccf, onesh=onesh, pc_pool=pc_pool, scr_pool=scr_pool
            )

            pending = []  # samples whose tail is deferred (2 deep)
            for pr in range(npair):
                # ---- load x pair (gram inputs only; no other PE deps) ----
                xa = xa_pool.tile([P, 2 * D], bf16, tag="xa")
                nc.sync.dma_start(xa[:], xal_d[pr, :, :])
                xblr = xb_pool.tile([MA, 4 * D], bf16, tag="xblr")
                nc.sync.dma_start(xblr[:], xblr_d[pr, :, :])

                for k in range(2):
                    # ---- upper-block gram + fused corrections ----
                    pg = pg_pool.tile([P, PGW], f32, tag="pg")
                    for r in range(NCH):
                        q = DCQ[r]
                        w = BW[r]
                        lsl = slice(k * D + P * r, k * D + P * (r + 1))
                        usl = slice(k * D + JB[r], (k + 1) * D)
                        nc.tensor.matmul(
                            pg[:, q : q + w], xa[:, lsl], xa[:, usl],
                            start=True, stop=False,
                        )
                        nc.tensor.matmul(
                            pg[:, q : q + w],
                            xblr[:, lsl],
                            xblr[:, 2 * D + k * D + JB[r] : 2 * D + (k + 1) * D],
                            start=False, stop=False,
                        )
                        # - gamma I on the diagonal block
                        dq = q + P * r - JB[r]
                        nc.tensor.matmul(
                            pg[:, dq : dq + P],
                            idc[:, 0:P],
                            idc[:, P : 2 * P],
                            start=False,
                            stop=True,
                        )

                    # ---- deferred tail (2 samples back) ----
                    if len(pending) == 2:
                        tail(pending.pop(0))
                    # ---- dcov = sqrt(pg*(-2e^t) + bias) -> fp16 packed ----
                    dc = dc_pool.tile([P, OW], f16, tag="dcov")
                    nc.scalar.activation(
                        dc[:], pg[:, 0:OW], AF.Sqrt, bias=bias_ap, scale=sc_ap
                    )
                    pending.append((dc, pr, k))
            for prev in pending:
                tail(prev)
            nc.sync.dma_start(ccf_d[:, :], ccf[:])

    # This walrus build accepts at most ONE sync wait per instruction.
    # Tile may attach several; hoist each extra wait onto its own no-op
    # placed just before the instruction (same engine, so ordering holds).
    if fixup:
        import bass_rust as _br

        for f in nc.m.functions:
            for blk in f.blocks:
                out_list = []
                changed = False
                for ins in blk.instructions:
                    si = getattr(ins, "sync_info", None)
                    if (
                        type(ins).__name__ != "InstNoOp"
                        and si is not None
                        and si.on_wait
                        and len(si.on_wait) > 1
                        and getattr(ins, "engine", None) is not None
                    ):
                        for j, w in enumerate(si.on_wait[:-1]):
                            nop = _br.InstNoOp(
                                name=f"I-w{j}-{ins.name}",
                                engine=ins.engine,
                                ins=[],
                                outs=[],
                            )
                            nop.sync_info = mybir.SyncInfo(
                                on_wait=[w], on_update=[]
                            )
                            out_list.append(nop)
                        ins.sync_info = mybir.SyncInfo(
                            on_wait=[si.on_wait[-1]], on_update=list(si.on_update)
                        )
                        changed = True
                    out_list.append(ins)
                if changed:
                    blk.instructions = out_list
    return nc


def make_consts(t_np):
    """Host-side constant tensors + the diagonal offset delta."""
    import ml_dtypes

    bf = ml_dtypes.bfloat16
    et = np.float32(np.exp(np.float32(np.asarray(t_np).reshape(-1)[0])))
    idc = np.zeros((P, 2 * P), dtype=np.float32)
    idc[:, 0:P] = np.eye(P)
    idc[:, P : 2 * P] = -GAMMA * np.eye(P)
    # diagonal argument of sqrt: 2*gamma*e^t + eps  (dpre_ii ~ 0)
    cval = np.float32(2.0 * GAMMA * et + EPS)
    delta = np.float32(np.sqrt(cval) - np.sqrt(np.float32(EPS)))
    return {
        "idc": idc.astype(bf),
        "onesh": np.ones((P, 1), dtype=np.float16),
    }, float(delta)


# triu assembly indices (static)
_TRIU_ROWSTART = np.zeros(D + 1, dtype=np.int64)
for _i in range(D):
    _TRIU_ROWSTART[_i + 1] = _TRIU_ROWSTART[_i] + (D - _i)
TRIU_LEN = int(_TRIU_ROWSTART[D])  # 131328


def pack_x(xc):
    """xc: [n, M, D] f32 -> bf16 pair-packed (xal, xblr)."""
    import ml_dtypes

    bf = ml_dtypes.bfloat16
    n = xc.shape[0]
    d = np.einsum("smd,smd->sd", xc, xc, dtype=np.float32)
    negh = (-0.5 * (d - KC)).astype(np.float32)
    xb16 = xc.astype(bf)

    def pairify(a):  # [n, R, D] -> [n/2, R, 2D]
        return np.ascontiguousarray(
            a.reshape(n // 2, 2, a.shape[1], D).transpose(0, 2, 1, 3)
        ).reshape(n // 2, a.shape[1], 2 * D)

    xal = pairify(xb16[:, 0:P, :])
    aug_l = np.empty((n, 2, D), dtype=bf)
    aug_l[:, 0, :] = 1.0
    aug_l[:, 1, :] = negh.astype(bf)
    aug_r = np.empty((n, 2, D), dtype=bf)
    aug_r[:, 0, :] = negh.astype(bf)
    aug_r[:, 1, :] = 1.0
    xmid = xb16[:, P:M, :]
    xbl = pairify(np.concatenate([xmid, aug_l], axis=1))
    xbr = pairify(np.concatenate([xmid, aug_r], axis=1))
    xblr = np.ascontiguousarray(np.concatenate([xbl, xbr], axis=2))
    return xal, xblr


def assemble(rect, ccf, delta):
    """rect: [npair,2,P,OW] f16, ccf: [P, 8*npair] raw rowsums -> cent."""
    npair = rect.shape[0]
    n = npair * 2
    d4 = rect.astype(np.float32).reshape(n, P, OW)
    # rs[s, 128r+p] = ccf[p, 8*pr+4*k+r]  (raw rowsums, gamma'd diagonal)
    rs = (
        ccf.reshape(P, npair, 2, NCH)
        .transpose(1, 2, 3, 0)
        .reshape(n, D)
        .astype(np.float64)
    )
    rs -= delta  # remove the gamma-shifted diagonal contribution
    tot = rs.sum(axis=1, keepdims=True) / (D * D)
    c = (rs / D - tot / 2).astype(np.float32)
    out = np.empty((n, TRIU_LEN), dtype=np.float32)
    for r in range(NCH):
        for p in range(P):
            i = P * r + p
            s = _TRIU_ROWSTART[i]
            ln = D - i
            q = DCQ[r] + i - JB[r]
            out[:, s : s + ln] = (
                d4[:, p, q : q + ln] - c[:, i : i + 1] - c[:, i:D]
            )
            # fix the gamma-shifted diagonal entry
            out[:, s] -= delta
    return out


def make_in_maps(x, t):
    consts, delta = make_consts(t)
    t128 = np.broadcast_to(
        np.asarray(t, dtype=np.float32).reshape(1, 1), (P, 1)
    ).copy()
    in_maps = []
    for c in range(NCORES):
        xal, xblr = pack_x(np.asarray(x[c * S : (c + 1) * S], dtype=np.float32))
        m = {"xal": xal, "xblr": xblr, "t128": t128}
        m.update(consts)
        in_maps.append(m)
    return in_maps, delta


_CACHE = {}


def kernel(**inputs):
    import concourse.bass_utils as bass_utils

    x = np.ascontiguousarray(inputs["x"], dtype=np.float32)
    t = np.asarray(inputs["t"], dtype=np.float32)
    assert x.shape == (B, M, D)

    if "nc" not in _CACHE:
        _CACHE["nc"] = build_nc(S)
    nc = _CACHE["nc"]

    in_maps, delta = make_in_maps(x, t)

    res = bass_utils.run_bass_kernel_spmd(nc, in_maps, core_ids=list(range(NCORES)))
    full = np.empty((B, TRIU_LEN), dtype=np.float32)
    for c in range(NCORES):
        full[c * S : (c + 1) * S] = assemble(
            res.results[c]["rect"], res.results[c]["ccf"], delta
        )
    return full
